# revision 6
# baseline (speedup 1.0000x reference)
"""Trainium2 Bass kernel for CorrespondenceFeatGeneration (patch-correlation argmax flow).

Math (per image, mirrors the reference):
  fin, fref: (256, 64, 64) -> unit-normalize each pixel across channels.
  corr[q, p] = <3x3 patch of fref at p, 3x3 patch of fin at q>   (2304-dim dot)
             = sum_{s in {0,1,2,64,65,66,128,129,130}} G[q+s, p+s],
               G[a, b] = <u_a, v_b>  (pixel correlation, K=256)
  max_idx[q] = argmax_{valid p} corr[q, p]  (first-max tie-break)

v2 structure (vs the dx-folded-into-PE baseline at ~317us; TimelineSim
makespan ~191us, verified bit-exact on HW):
  PE computes G ONCE (fp16 hi/lo 3-term split, fp32 PSUM) -> 3x less PE work.
  The 9-shift sum is assembled hierarchically on the other engines:
    C3 = G + G^(+1,+1) + G^(+2,+2)   (dx level)
    corr = C3 + C3^(+64,+64) + C3^(+128,+128)   (dy level)
  Column(+s) shifts are free AP views. Partition(+s) shifts:
    +1/+2: SDMA partition-rebase stages (SBUF->SBUF, contiguous rows,
           split L/R at the PSUM-piece boundary for early starts)
    +64:   ACT partition-rebase copies (HW-probed legal for 64-aligned
           windows; engines CANNOT rebase by non-32-aligned offsets, and
           tensor_tensor requires equal input partition bases -- probed)
    +128:  tile-aligned free view (next C3 tile)
  C3 rows 126,127 of every tile only feed outputs with qx in {62,63}
  (invalid, host-discarded), so the shift stages have NO cross-tile deps:
  rows 126/127 are junk-filled from same-tile rows to stay finite.
  corr is materialized px-compacted over valid p (px<62, py<62 -> 3844
  wide); max/max_index scan it; host decodes idx via divmod 62.
  Pipeline notes (why it hits ~17us/tile steady state, DVE+Pool ~98% busy):
   - G in 4 PSUM pieces (2 banks each); ACT copies each piece to SBUF right
     after its 12 matmuls so the PE almost never stalls (p-state stays hot).
   - assembly runs with a 2-TILE LAG so no engine FIFO ever head-of-line
     blocks on a just-produced c3 (every input is >=1 iteration old).
   - s2 stage DMAs issue on the ACT HWDGE ring (nc.scalar) so their waits
     do not block the SP ring; s2/st64 gate on events that are past anyway.
   - engine split per tile: Pool: a1 L/R + t2-part; DVE: a2 L/R, corr-part,
     max, max_index; ACT: 8 psum copies + stage64; DMA: 4 stage + 4 junk.
  tensor_tensor_reduce (fused add+max) hard-crashes the device in this
  toolchain -- do not use. SWDGE (nc.gpsimd.dma_start) silently no-ops
  under the axon/PJRT path -- do not use.

Device strategy (8 NeuronCores, SPMD): shard q: 2 images x 4 blocks of 1024.
Host: unit-normalize, fp16 split, pad; decode idx -> flow + 9 shifts.
"""

import numpy as np

H = W = 64
C = 256
HP = H * W          # 4096 pixel positions per image
PW = 4096           # G/ref pixel columns (exact)
QW_PAD = 4352       # padded input pixel columns for windowing
QBLK = 1024         # q positions per core
QWIN = 1280         # per-core input window width
NQT = 8             # output q-tiles of 128 per core
NCT = 9             # C3/G tiles per core (dy lookahead)
W3 = 4094           # C3 width (cols 0..4093)
NV = 62 * 62        # compact valid-p width (py<62, px<62)

_RUNNER = None


def _build_runner():
    import sys
    if '/opt/trn_rl_repo' not in sys.path:
        sys.path.insert(0, '/opt/trn_rl_repo')
    import concourse.bass as bass  # noqa: F401
    import concourse.tile as tile
    from concourse import bacc, mybir
    from concourse.bass_utils import run_bass_kernel_spmd

    f16 = mybir.dt.float16
    f32 = mybir.dt.float32
    u32 = mybir.dt.uint32

    nc = bacc.Bacc("TRN2", target_bir_lowering=False, debug=False, num_devices=8)

    d_in = {}
    for nm in ("uin_hi", "uin_lo"):
        d_in[nm] = nc.dram_tensor(nm, [C, QWIN], f16, kind="ExternalInput").ap()
    for nm in ("uref_hi", "uref_lo"):
        d_in[nm] = nc.dram_tensor(nm, [C, PW], f16, kind="ExternalInput").ap()
    idx_d = nc.dram_tensor("idx", [128, NQT], u32, kind="ExternalOutput").ap()
    junk_d = nc.dram_tensor("junk", [1, 8], f32, kind="ExternalOutput").ap()

    TERMS = [("uin_hi", "uref_hi"), ("uin_lo", "uref_hi"), ("uin_hi", "uref_lo")]

    def cview(t, py0):
        # [128, 62, 62] view of a [128, 4096] tile: cols 64*(py0+py)+px
        return t[:, 64 * py0: 64 * py0 + 3968].rearrange(
            "p (a b) -> p a b", b=64)[:, :, 0:62]

    with tile.TileContext(nc) as tc:
        with tc.tile_pool(name="const", bufs=1) as cpool, \
             tc.tile_pool(name="work2", bufs=2) as wpool, \
             tc.tile_pool(name="small", bufs=2) as spool, \
             tc.tile_pool(name="stage", bufs=1) as stpool, \
             tc.tile_pool(name="ps", bufs=1, space="PSUM") as ps:

            # Input DMAs; order matters for the wait-absorber below.
            # uref loads split by column half; all LEFT halves load first so
            # PE pieces 0-1 (all three terms) unblock as early as possible.
            ins = {}
            for nm, w in (("uin_hi", QWIN), ("uref_hi", PW),
                          ("uin_lo", QWIN), ("uref_lo", PW)):
                for ch in range(2):
                    t = cpool.tile([128, w], f16, name="dma_in", tag=f"{nm}{ch}")
                    ins[(nm, ch)] = t
                    if w == QWIN:
                        nc.sync.dma_start(t[:], d_in[nm][128 * ch:128 * (ch + 1), :])
                    else:
                        nc.sync.dma_start(t[:, 0:2048],
                                          d_in[nm][128 * ch:128 * (ch + 1), 0:2048])
            for nm in ("uref_hi", "uref_lo"):
                for ch in range(2):
                    nc.sync.dma_start(ins[(nm, ch)][:, 2048:PW],
                                      d_in[nm][128 * ch:128 * (ch + 1), 2048:PW])

            # Wait-absorber matmuls: walrus allows only one sync wait on the
            # LDW side of a matmul; absorb the uin-side DMA waits here so real
            # matmuls never carry two fresh DMA waits.
            junk_ps = ps.tile([128, 8], f32, name="junkps", tag="pspc3")
            regions = [ins[(nm, ch)]
                       for nm in ("uin_hi", "uin_lo")
                       for ch in range(2)]
            for i, r in enumerate(regions):
                nc.tensor.matmul(junk_ps[:1, :8], r[:, :1], r[:, :8],
                                 start=(i == 0), stop=(i == len(regions) - 1))
            junk_sb = stpool.tile([128, 8], f32, name="junksb", tag="junk")
            nc.vector.tensor_copy(junk_sb[:1, :8], junk_ps[:1, :8])

            idx_stage = stpool.tile([128, NQT], u32, name="idxs", tag="idxs")

            def assemble_main(t, prev, cur, mx, mi):
                # stage64[q] = C3[q+64, p+64], compact px, ACT rebase copies
                st64 = wpool.tile([128, NV], f32, name="st64", tag="st64")
                nc.scalar.copy(st64[0:64, :], cview(prev, 1)[64:128])
                nc.scalar.copy(st64[64:128, :], cview(cur, 1)[0:64])
                # t2 = C3[compact] + stage64 (in-place onto st64);
                # corr: t2 += C3_next[py+2 view]. Pool/DVE work concurrently.
                t2 = st64
                SP = 40 * 62   # 40 of 62 py-groups on Pool for t2
                SC = 40 * 62   # 40 of 62 py-groups on DVE for corr
                nc.gpsimd.tensor_add(t2[:, 0:SP], cview(prev, 0)[:, 0:40, :],
                                     st64[:, 0:SP])
                nc.vector.tensor_add(t2[:, SP:], cview(prev, 0)[:, 40:62, :],
                                     st64[:, SP:])
                nc.vector.tensor_add(t2[:, 0:SC], t2[:, 0:SC],
                                     cview(cur, 2)[:, 0:40, :])
                nc.gpsimd.tensor_add(t2[:, SC:], t2[:, SC:],
                                     cview(cur, 2)[:, 40:62, :])
                nc.vector.max(mx[:], t2[:])
                nc.vector.max_index(mi[:], mx[:], t2[:])
                nc.vector.tensor_copy(idx_stage[:, t:t + 1], mi[:, 0:1])

            c3_tiles = {}
            for ct in range(NCT):
                # --- PE: G tile ct in PSUM, four 2-bank pieces; ACT copies
                # each piece to SBUF right after its matmuls so the next
                # tile's PE piece is never blocked long (keeps PE p-state hot).
                gsb = wpool.tile([128, PW], f32, name="gsb", tag="gsb")
                for pc in range(4):
                    psp = ps.tile([128, 1024], f32, name=f"pspc{pc}",
                                  tag=f"pspc{pc}")
                    step = 0
                    for (anm, bnm) in TERMS:
                        for ch in range(2):
                            lhsT = ins[(anm, ch)][:, ct * 128: ct * 128 + 128]
                            for bk in range(2):
                                rhs = ins[(bnm, ch)][:, pc * 1024 + bk * 512:
                                                     pc * 1024 + bk * 512 + 512]
                                nc.tensor.matmul(
                                    psp[:, bk * 512:(bk + 1) * 512], lhsT, rhs,
                                    start=(step == 0), stop=(step == 5))
                            step += 1
                    for bk in range(2):
                        nc.scalar.copy(
                            gsb[:, pc * 1024 + bk * 512: pc * 1024 + (bk + 1) * 512],
                            psp[:, bk * 512:(bk + 1) * 512])

                # --- shift stages via DMA rebase, split L/R at the gsb
                # piece boundary so each half starts as soon as its source
                # pieces land: s1 = G^(+1,+1), s2 = G^(+2,+2).
                # Rows 126/127 junk (feed only qx in {62,63}, host-discarded).
                s1L = wpool.tile([128, 1984], f32, name="s1L", tag="s1L")
                s1R = wpool.tile([128, 2112], f32, name="s1R", tag="s1R")
                s2L = wpool.tile([128, 1984], f32, name="s2L", tag="s2L", bufs=1)
                s2R = wpool.tile([128, 2112], f32, name="s2R", tag="s2R", bufs=1)
                nc.sync.dma_start(s1L[0:127, :], gsb[1:128, 1:1985])
                nc.sync.dma_start(s1L[127:128, :], gsb[127:128, 1:1985])
                nc.scalar.dma_start(s2L[0:126, :], gsb[2:128, 2:1986])
                nc.scalar.dma_start(s2L[126:128, :], gsb[126:128, 2:1986])
                nc.sync.dma_start(s1R[0:127, 0:2110], gsb[1:128, 1985:4095])
                nc.sync.dma_start(s1R[127:128, 0:2110], gsb[127:128, 1985:4095])
                nc.scalar.dma_start(s2R[0:126, 0:2110], gsb[2:128, 1986:4096])
                nc.scalar.dma_start(s2R[126:128, 0:2110], gsb[126:128, 1986:4096])

                # --- a1 (GPSIMD) L/R: c3 = G + s1;  a2 (DVE, in-place) += s2
                # Strided over valid px (<62) only: C3's px 62/63 columns are
                # never read downstream, so skip them (3% fewer elements).
                # L/R split at 1984 (31/33 py-groups, 64-aligned).
                c3 = wpool.tile([128, PW], f32, name="c3", tag="c3", bufs=3)

                def pxv(t_, c0, ng):
                    return t_[:, c0:c0 + ng * 64].rearrange(
                        "p (a b) -> p a b", b=64)[:, :, 0:62]

                nc.gpsimd.tensor_add(pxv(c3, 0, 31), pxv(gsb, 0, 31),
                                     pxv(s1L, 0, 31))
                nc.vector.tensor_add(pxv(c3, 0, 31), pxv(c3, 0, 31),
                                     pxv(s2L, 0, 31))
                nc.gpsimd.tensor_add(pxv(c3, 1984, 33), pxv(gsb, 1984, 33),
                                     pxv(s1R, 0, 33))
                nc.vector.tensor_add(pxv(c3, 1984, 33), pxv(c3, 1984, 33),
                                     pxv(s2R, 0, 33))
                c3_tiles[ct] = c3
                # 2-tile-lag assembly: every input of each queued op is >=1
                # iteration old, so no engine FIFO head-of-line blocks.
                if ct < 2:
                    continue
                t = ct - 2
                mx = spool.tile([128, 8], f32, name="mx", tag="mx")
                mi = spool.tile([128, 8], u32, name="mi", tag="mi")
                assemble_main(t, c3_tiles[t], c3_tiles[t + 1], mx, mi)
                del c3_tiles[t]

            mx = spool.tile([128, 8], f32, name="mxf", tag="mx")
            mi = spool.tile([128, 8], u32, name="mif", tag="mi")
            assemble_main(NCT - 2, c3_tiles[NCT - 2], c3_tiles[NCT - 1], mx, mi)

            nc.sync.dma_start(idx_d[:], idx_stage[:])
            nc.sync.dma_start(junk_d[:], junk_sb[:1, :8])

    nc.compile()
    return nc, run_bass_kernel_spmd


def _unit_pixels(f):
    # f: (C, H, W) float32; unit L2 norm per pixel across channels (fp32 math)
    n = np.sqrt(np.sum(f * f, axis=0, keepdims=True, dtype=np.float32))
    return (f / np.maximum(n, np.float32(1e-12))).astype(np.float32)


def _split_f16(a):
    hi = a.astype(np.float16)
    lo = (a - hi.astype(np.float32)).astype(np.float16)
    return hi, lo


def kernel(dense_features1, dense_features2, img_ref_hr):
    global _RUNNER
    if _RUNNER is None:
        _RUNNER = _build_runner()
    nc, run_spmd = _RUNNER

    f1 = np.asarray(dense_features1, dtype=np.float32)  # input features (b,C,H,W)
    f2 = np.asarray(dense_features2, dtype=np.float32)  # ref features
    B = f1.shape[0]
    assert B == 2 and f1.shape[1:] == (C, H, W)

    in_maps = []
    per_img = []
    for b in range(B):
        fin_u = _unit_pixels(f1[b]).reshape(C, HP)
        fref_u = _unit_pixels(f2[b]).reshape(C, HP)
        uin = np.zeros((C, QW_PAD), np.float32)
        uin[:, :HP] = fin_u
        uref = fref_u
        uin_hi, uin_lo = _split_f16(uin)
        uref_hi, uref_lo = _split_f16(uref)
        per_img.append((uin_hi, uin_lo, uref_hi, uref_lo))

    for core in range(8):
        b, qblk = divmod(core, 4)
        uin_hi, uin_lo, uref_hi, uref_lo = per_img[b]
        q0 = qblk * QBLK
        in_maps.append({
            "uin_hi": np.ascontiguousarray(uin_hi[:, q0:q0 + QWIN]),
            "uin_lo": np.ascontiguousarray(uin_lo[:, q0:q0 + QWIN]),
            "uref_hi": np.ascontiguousarray(uref_hi),
            "uref_lo": np.ascontiguousarray(uref_lo),
        })

    results = run_spmd(nc, in_maps, list(range(8))).results

    # Decode: idx_stage[part, tile] = argmax over compact (py, px) grid for
    # q_local = tile*128 + part, global q = core_q0 + q_local.
    out = np.zeros((B, 9, H, W, 2), np.float32)
    qx_grid = np.arange(62, dtype=np.float32)[None, :]
    qy_grid = np.arange(62, dtype=np.float32)[:, None]
    for b in range(B):
        idx_full = np.zeros(HP, np.int64)
        for qblk in range(4):
            r = results[b * 4 + qblk]["idx"]  # (128, NQT) uint32
            idx_full[qblk * QBLK:(qblk + 1) * QBLK] = r.T.reshape(-1)
        idx_grid = idx_full.reshape(H, W)[:62, :62]
        py = (idx_grid // 62).astype(np.float32)
        px = (idx_grid % 62).astype(np.float32)
        flow = np.zeros((H, W, 2), np.float32)
        flow[:62, :62, 0] = px - qx_grid
        flow[:62, :62, 1] = py - qy_grid
        for k, (i, j) in enumerate([(i, j) for i in range(3) for j in range(3)]):
            out[b, k, i:, j:, :] = flow[:H - i, :W - j, :]
    return out


# revision 7
# speedup vs baseline: 1.0142x; 1.0142x over previous
"""Trainium2 Bass kernel for CorrespondenceFeatGeneration (patch-correlation argmax flow).

Math (per image, mirrors the reference):
  fin, fref: (256, 64, 64) -> unit-normalize each pixel across channels.
  corr[q, p] = <3x3 patch of fref at p, 3x3 patch of fin at q>   (2304-dim dot)
             = sum_{s in {0,1,2,64,65,66,128,129,130}} G[q+s, p+s],
               G[a, b] = <u_a, v_b>  (pixel correlation, K=256)
  max_idx[q] = argmax_{valid p} corr[q, p]  (first-max tie-break)

v2 structure (vs the dx-folded-into-PE baseline at ~317us; TimelineSim
makespan ~189us, verified bit-exact on HW):
  PE computes G ONCE (fp16 hi/lo 3-term split, fp32 PSUM) -> 3x less PE work.
  The 9-shift sum is assembled hierarchically on the other engines:
    C3 = G + G^(+1,+1) + G^(+2,+2)   (dx level)
    corr = C3 + C3^(+64,+64) + C3^(+128,+128)   (dy level)
  Column(+s) shifts are free AP views. Partition(+s) shifts:
    +1/+2: SDMA partition-rebase stages (SBUF->SBUF, contiguous rows,
           split L/R at the PSUM-piece boundary for early starts)
    +64:   ACT partition-rebase copies (HW-probed legal for 64-aligned
           windows; engines CANNOT rebase by non-32-aligned offsets, and
           tensor_tensor requires equal input partition bases -- probed)
    +128:  tile-aligned free view (next C3 tile)
  C3 rows 126,127 of every tile only feed outputs with qx in {62,63}
  (invalid, host-discarded), so the shift stages have NO cross-tile deps:
  rows 126/127 are junk-filled from same-tile rows to stay finite.
  corr is materialized px-compacted over valid p (px<62, py<62 -> 3844
  wide); max/max_index scan it; host decodes idx via divmod 62.
  Pipeline notes (why it hits ~17us/tile steady state, DVE+Pool ~98% busy):
   - G in 4 PSUM pieces (2 banks each); ACT copies each piece to SBUF right
     after its 12 matmuls so the PE almost never stalls (p-state stays hot).
   - assembly runs with a 2-TILE LAG so no engine FIFO ever head-of-line
     blocks on a just-produced c3 (every input is >=1 iteration old).
   - s2 stage DMAs issue on the ACT HWDGE ring (nc.scalar) so their waits
     do not block the SP ring; s2/st64 gate on events that are past anyway.
   - engine split per tile: Pool: a1 L/R + t2-part; DVE: a2 L/R, corr-part,
     max, max_index; ACT: 8 psum copies + stage64; DMA: 4 stage + 4 junk.
   - a1/a2 run on strided valid-px views (skip C3's never-read px 62/63).
  tensor_tensor_reduce (fused add+max) hard-crashes the device in this
  toolchain -- do not use. SWDGE (nc.gpsimd.dma_start) silently no-ops
  under the axon/PJRT path -- do not use.

Device strategy (8 NeuronCores, SPMD): shard q: 2 images x 4 blocks of 1024.
Host: unit-normalize, fp16 split, pad; decode idx -> flow + 9 shifts.
"""

import numpy as np

H = W = 64
C = 256
HP = H * W          # 4096 pixel positions per image
PW = 4096           # G/ref pixel columns (exact)
QW_PAD = 4352       # padded input pixel columns for windowing
QBLK = 1024         # q positions per core
QWIN = 1280         # per-core input window width
NQT = 8             # output q-tiles of 128 per core
NCT = 9             # C3/G tiles per core (dy lookahead)
W3 = 4094           # C3 width (cols 0..4093)
NV = 62 * 62        # compact valid-p width (py<62, px<62)

_RUNNER = None


def _build_runner():
    import sys
    if '/opt/trn_rl_repo' not in sys.path:
        sys.path.insert(0, '/opt/trn_rl_repo')
    import concourse.bass as bass  # noqa: F401
    import concourse.tile as tile
    from concourse import bacc, mybir
    from concourse.bass_utils import run_bass_kernel_spmd

    f16 = mybir.dt.float16
    f32 = mybir.dt.float32
    u32 = mybir.dt.uint32

    nc = bacc.Bacc("TRN2", target_bir_lowering=False, debug=False, num_devices=8)

    d_in = {}
    for nm in ("uin_hi", "uin_lo"):
        d_in[nm] = nc.dram_tensor(nm, [C, QWIN], f16, kind="ExternalInput").ap()
    for nm in ("uref_hi", "uref_lo"):
        d_in[nm] = nc.dram_tensor(nm, [C, PW], f16, kind="ExternalInput").ap()
    idx_d = nc.dram_tensor("idx", [128, NQT], u32, kind="ExternalOutput").ap()
    junk_d = nc.dram_tensor("junk", [1, 8], f32, kind="ExternalOutput").ap()

    TERMS = [("uin_hi", "uref_hi"), ("uin_lo", "uref_hi"), ("uin_hi", "uref_lo")]

    def cview(t, py0):
        # [128, 62, 62] view of a [128, 4096] tile: cols 64*(py0+py)+px
        return t[:, 64 * py0: 64 * py0 + 3968].rearrange(
            "p (a b) -> p a b", b=64)[:, :, 0:62]

    with tile.TileContext(nc) as tc:
        with tc.tile_pool(name="const", bufs=1) as cpool, \
             tc.tile_pool(name="work2", bufs=2) as wpool, \
             tc.tile_pool(name="small", bufs=2) as spool, \
             tc.tile_pool(name="stage", bufs=1) as stpool, \
             tc.tile_pool(name="ps", bufs=1, space="PSUM") as ps:

            # Input DMAs; order matters for the wait-absorber below.
            # uref loads split by column half; all LEFT halves load first so
            # PE pieces 0-1 (all three terms) unblock as early as possible.
            ins = {}
            for nm, w in (("uin_hi", QWIN), ("uref_hi", PW),
                          ("uin_lo", QWIN), ("uref_lo", PW)):
                for ch in range(2):
                    t = cpool.tile([128, w], f16, name="dma_in", tag=f"{nm}{ch}")
                    ins[(nm, ch)] = t
                    if w == QWIN:
                        nc.sync.dma_start(t[:], d_in[nm][128 * ch:128 * (ch + 1), :])
                    else:
                        nc.sync.dma_start(t[:, 0:2048],
                                          d_in[nm][128 * ch:128 * (ch + 1), 0:2048])
            for nm in ("uref_hi", "uref_lo"):
                for ch in range(2):
                    nc.sync.dma_start(ins[(nm, ch)][:, 2048:PW],
                                      d_in[nm][128 * ch:128 * (ch + 1), 2048:PW])

            # Wait-absorber matmuls: walrus allows only one sync wait on the
            # LDW side of a matmul; absorb the uin-side DMA waits here so real
            # matmuls never carry two fresh DMA waits.
            junk_ps = ps.tile([128, 8], f32, name="junkps", tag="pspc3")
            regions = [ins[(nm, ch)]
                       for nm in ("uin_hi", "uin_lo")
                       for ch in range(2)]
            for i, r in enumerate(regions):
                nc.tensor.matmul(junk_ps[:1, :8], r[:, :1], r[:, :8],
                                 start=(i == 0), stop=(i == len(regions) - 1))
            junk_sb = stpool.tile([128, 8], f32, name="junksb", tag="junk")
            nc.vector.tensor_copy(junk_sb[:1, :8], junk_ps[:1, :8])

            idx_stage = stpool.tile([128, NQT], u32, name="idxs", tag="idxs")

            def assemble_main(t, prev, cur, mx, mi):
                # stage64[q] = C3[q+64, p+64], compact px, ACT rebase copies
                st64 = wpool.tile([128, NV], f32, name="st64", tag="st64")
                nc.scalar.copy(st64[0:64, :], cview(prev, 1)[64:128])
                nc.scalar.copy(st64[64:128, :], cview(cur, 1)[0:64])
                # t2 = C3[compact] + stage64 (in-place onto st64);
                # corr: t2 += C3_next[py+2 view]. Pool/DVE work concurrently.
                t2 = st64
                SP = 40 * 62   # 40 of 62 py-groups on Pool for t2
                SC = 40 * 62   # 40 of 62 py-groups on DVE for corr
                nc.gpsimd.tensor_add(t2[:, 0:SP], cview(prev, 0)[:, 0:40, :],
                                     st64[:, 0:SP])
                nc.vector.tensor_add(t2[:, SP:], cview(prev, 0)[:, 40:62, :],
                                     st64[:, SP:])
                nc.vector.tensor_add(t2[:, 0:SC], t2[:, 0:SC],
                                     cview(cur, 2)[:, 0:40, :])
                nc.gpsimd.tensor_add(t2[:, SC:], t2[:, SC:],
                                     cview(cur, 2)[:, 40:62, :])
                nc.vector.max(mx[:], t2[:])
                nc.vector.max_index(mi[:], mx[:], t2[:])
                nc.vector.tensor_copy(idx_stage[:, t:t + 1], mi[:, 0:1])

            c3_tiles = {}
            for ct in range(NCT):
                # --- PE: G tile ct in PSUM, four 2-bank pieces; ACT copies
                # each piece to SBUF right after its matmuls so the next
                # tile's PE piece is never blocked long (keeps PE p-state hot).
                gsb = wpool.tile([128, PW], f32, name="gsb", tag="gsb")
                for pc in range(4):
                    psp = ps.tile([128, 1024], f32, name=f"pspc{pc}",
                                  tag=f"pspc{pc}")
                    step = 0
                    for (anm, bnm) in TERMS:
                        for ch in range(2):
                            lhsT = ins[(anm, ch)][:, ct * 128: ct * 128 + 128]
                            for bk in range(2):
                                rhs = ins[(bnm, ch)][:, pc * 1024 + bk * 512:
                                                     pc * 1024 + bk * 512 + 512]
                                nc.tensor.matmul(
                                    psp[:, bk * 512:(bk + 1) * 512], lhsT, rhs,
                                    start=(step == 0), stop=(step == 5))
                            step += 1
                    for bk in range(2):
                        nc.scalar.copy(
                            gsb[:, pc * 1024 + bk * 512: pc * 1024 + (bk + 1) * 512],
                            psp[:, bk * 512:(bk + 1) * 512])

                # --- shift stages via DMA rebase, split L/R at the gsb
                # piece boundary so each half starts as soon as its source
                # pieces land: s1 = G^(+1,+1), s2 = G^(+2,+2).
                # Rows 126/127 junk (feed only qx in {62,63}, host-discarded).
                s1L = wpool.tile([128, 1984], f32, name="s1L", tag="s1L")
                s1R = wpool.tile([128, 2112], f32, name="s1R", tag="s1R")
                s2L = wpool.tile([128, 1984], f32, name="s2L", tag="s2L", bufs=1)
                s2R = wpool.tile([128, 2112], f32, name="s2R", tag="s2R", bufs=1)
                nc.sync.dma_start(s1L[0:127, :], gsb[1:128, 1:1985])
                nc.sync.dma_start(s1L[127:128, :], gsb[127:128, 1:1985])
                nc.scalar.dma_start(s2L[0:126, :], gsb[2:128, 2:1986])
                nc.scalar.dma_start(s2L[126:128, :], gsb[126:128, 2:1986])
                nc.sync.dma_start(s1R[0:127, 0:2110], gsb[1:128, 1985:4095])
                nc.sync.dma_start(s1R[127:128, 0:2110], gsb[127:128, 1985:4095])
                nc.scalar.dma_start(s2R[0:126, 0:2110], gsb[2:128, 1986:4096])
                nc.scalar.dma_start(s2R[126:128, 0:2110], gsb[126:128, 1986:4096])

                # --- a1 (GPSIMD) L/R: c3 = G + s1;  a2 (DVE, in-place) += s2
                # Strided over valid px (<62) only: C3's px 62/63 columns are
                # never read downstream, so skip them (3% fewer elements).
                # L/R split at 1984 (31/33 py-groups, 64-aligned).
                c3 = wpool.tile([128, PW], f32, name="c3", tag="c3", bufs=3)

                def pxv(t_, c0, ng):
                    return t_[:, c0:c0 + ng * 64].rearrange(
                        "p (a b) -> p a b", b=64)[:, :, 0:62]

                nc.gpsimd.tensor_add(pxv(c3, 0, 31), pxv(gsb, 0, 31),
                                     pxv(s1L, 0, 31))
                nc.vector.tensor_add(pxv(c3, 0, 31), pxv(c3, 0, 31),
                                     pxv(s2L, 0, 31))
                nc.gpsimd.tensor_add(pxv(c3, 1984, 33), pxv(gsb, 1984, 33),
                                     pxv(s1R, 0, 33))
                nc.vector.tensor_add(pxv(c3, 1984, 33), pxv(c3, 1984, 33),
                                     pxv(s2R, 0, 33))
                c3_tiles[ct] = c3
                # 2-tile-lag assembly: every input of each queued op is >=1
                # iteration old, so no engine FIFO head-of-line blocks.
                if ct < 2:
                    continue
                t = ct - 2
                mx = spool.tile([128, 8], f32, name="mx", tag="mx")
                mi = spool.tile([128, 8], u32, name="mi", tag="mi")
                assemble_main(t, c3_tiles[t], c3_tiles[t + 1], mx, mi)
                del c3_tiles[t]

            mx = spool.tile([128, 8], f32, name="mxf", tag="mx")
            mi = spool.tile([128, 8], u32, name="mif", tag="mi")
            assemble_main(NCT - 2, c3_tiles[NCT - 2], c3_tiles[NCT - 1], mx, mi)

            nc.sync.dma_start(idx_d[:], idx_stage[:])
            nc.sync.dma_start(junk_d[:], junk_sb[:1, :8])

    nc.compile()
    return nc, run_bass_kernel_spmd


def _unit_pixels(f):
    # f: (C, H, W) float32; unit L2 norm per pixel across channels (fp32 math)
    n = np.sqrt(np.sum(f * f, axis=0, keepdims=True, dtype=np.float32))
    return (f / np.maximum(n, np.float32(1e-12))).astype(np.float32)


def _split_f16(a):
    hi = a.astype(np.float16)
    lo = (a - hi.astype(np.float32)).astype(np.float16)
    return hi, lo


def kernel(dense_features1, dense_features2, img_ref_hr):
    global _RUNNER
    if _RUNNER is None:
        _RUNNER = _build_runner()
    nc, run_spmd = _RUNNER

    f1 = np.asarray(dense_features1, dtype=np.float32)  # input features (b,C,H,W)
    f2 = np.asarray(dense_features2, dtype=np.float32)  # ref features
    B = f1.shape[0]
    assert B == 2 and f1.shape[1:] == (C, H, W)

    in_maps = []
    per_img = []
    for b in range(B):
        fin_u = _unit_pixels(f1[b]).reshape(C, HP)
        fref_u = _unit_pixels(f2[b]).reshape(C, HP)
        uin = np.zeros((C, QW_PAD), np.float32)
        uin[:, :HP] = fin_u
        uref = fref_u
        uin_hi, uin_lo = _split_f16(uin)
        uref_hi, uref_lo = _split_f16(uref)
        per_img.append((uin_hi, uin_lo, uref_hi, uref_lo))

    for core in range(8):
        b, qblk = divmod(core, 4)
        uin_hi, uin_lo, uref_hi, uref_lo = per_img[b]
        q0 = qblk * QBLK
        in_maps.append({
            "uin_hi": np.ascontiguousarray(uin_hi[:, q0:q0 + QWIN]),
            "uin_lo": np.ascontiguousarray(uin_lo[:, q0:q0 + QWIN]),
            "uref_hi": np.ascontiguousarray(uref_hi),
            "uref_lo": np.ascontiguousarray(uref_lo),
        })

    results = run_spmd(nc, in_maps, list(range(8))).results

    # Decode: idx_stage[part, tile] = argmax over compact (py, px) grid for
    # q_local = tile*128 + part, global q = core_q0 + q_local.
    out = np.zeros((B, 9, H, W, 2), np.float32)
    qx_grid = np.arange(62, dtype=np.float32)[None, :]
    qy_grid = np.arange(62, dtype=np.float32)[:, None]
    for b in range(B):
        idx_full = np.zeros(HP, np.int64)
        for qblk in range(4):
            r = results[b * 4 + qblk]["idx"]  # (128, NQT) uint32
            idx_full[qblk * QBLK:(qblk + 1) * QBLK] = r.T.reshape(-1)
        idx_grid = idx_full.reshape(H, W)[:62, :62]
        py = (idx_grid // 62).astype(np.float32)
        px = (idx_grid % 62).astype(np.float32)
        flow = np.zeros((H, W, 2), np.float32)
        flow[:62, :62, 0] = px - qx_grid
        flow[:62, :62, 1] = py - qy_grid
        for k, (i, j) in enumerate([(i, j) for i in range(3) for j in range(3)]):
            out[b, k, i:, j:, :] = flow[:H - i, :W - j, :]
    return out
